# revision 2
# baseline (speedup 1.0000x reference)
"""GroupFC kernel for Trainium2, data-parallel across 8 NeuronCores.

Problem: out = data @ W.T + b
  data: [32768, 1024] f32, W: [1024, 1024] f32 (block-diagonal-masked), b: [1024] f32

Strategy:
  - Shard batch dim across 8 cores (4096 rows each); replicate W, b.
  - Host-side: cast data shard + W to bf16, pre-transpose so the contraction
    dim (in_features) lands on SBUF partitions; broadcast b to [128, 1024].
  - On-chip per core: the whole 8 MiB bf16 data shard is SBUF-resident as
    chunk tiles (fine-grained deps let the PE start as soon as the first
    chunks land). Per 128-row batch sub-tile, each 512-wide output half is
    accumulated over 8 K-tiles in its own PSUM bank as a contiguous k-run,
    so the first half's bias-add + store overlaps the second half's matmuls.
    Bias is added during PSUM->SBUF evacuation on DVE with bf16 output
    (halves store traffic; host upcasts to f32), stores go out in natural
    [batch, out] layout.
  - Kernel span is PE-bound: 512 N=512 matmuls x ~216 ns. Head: warm-up
    matmuls on a zeroed scratch tile keep the PE HAM clock busy while the
    primer DMAs (spread over sync/scalar/gpsimd queues) land. Tail: the
    last sub's final half is evacuated/stored as two 256-col pieces on both
    HWDGE queues.
"""

import os
import sys
from contextlib import ExitStack

import numpy as np

try:
    import concourse.bass as bass  # noqa: F401
except ImportError:
    sys.path.insert(0, "/opt/trn_rl_repo")

import ml_dtypes

import concourse.tile as tile
from concourse import bacc, mybir
from concourse.bass_utils import run_bass_kernel_spmd

N_CORES = 8
BATCH = 32768
SHARD = BATCH // N_CORES  # 4096
IN_DIM = 1024
OUT_DIM = 1024
P = 128
KT = IN_DIM // P  # 8 contraction tiles
NFREE = 512  # psum bank free-dim (fp32)
CCHUNK = 1024  # batch columns per data chunk tile
NCHUNKS = SHARD // CCHUNK  # 4
SUBS_PER_CHUNK = CCHUNK // P  # 8
NSUBS = SHARD // P  # 32

_CACHE = {}


def _build():
    nc = bacc.Bacc("TRN2", target_bir_lowering=False, debug=False)
    dT = nc.dram_tensor(
        "dT", [IN_DIM, SHARD], mybir.dt.bfloat16, kind="ExternalInput"
    ).ap()
    wT = nc.dram_tensor(
        "wT", [IN_DIM, OUT_DIM], mybir.dt.bfloat16, kind="ExternalInput"
    ).ap()
    biasb = nc.dram_tensor(
        "biasb", [P, OUT_DIM], mybir.dt.float32, kind="ExternalInput"
    ).ap()
    out = nc.dram_tensor(
        "out", [SHARD, OUT_DIM], mybir.dt.bfloat16, kind="ExternalOutput"
    ).ap()

    with tile.TileContext(nc) as tc:
        with ExitStack() as ctx:
            wp = ctx.enter_context(tc.tile_pool(name="w", bufs=1))
            bp = ctx.enter_context(tc.tile_pool(name="bias", bufs=1))
            dp = ctx.enter_context(tc.tile_pool(name="d", bufs=1))
            pp = ctx.enter_context(tc.tile_pool(name="psum", bufs=4, space="PSUM"))
            op = ctx.enter_context(tc.tile_pool(name="o", bufs=6))

            # Scratch for PE warm-up: memset is the first DVE op so dummy
            # matmuls can start right after the framework preamble.
            scratch = wp.tile([P, P], mybir.dt.bfloat16, tag="warm_scratch")
            nc.vector.memset(scratch[:], 0)

            # w_tiles[k][nh]: [128, 512] halves of wT k-tile.
            w_tiles = [[None] * 2 for _ in range(KT)]
            # d0a/d0b: first chunk split as two [128, 512] tiles (subs 0-3 /
            # 4-7); d_tiles[k][c] for c>=1: [128, 1024] chunks (8 subs each).
            d0 = [[None] * 2 for _ in range(KT)]
            d_tiles = [[None] * NCHUNKS for _ in range(KT)]

            def load_w(eng, k, j):
                wt = wp.tile([P, NFREE], mybir.dt.bfloat16, tag=f"w{k}_{j}")
                eng.dma_start(
                    out=wt[:],
                    in_=wT[k * P : (k + 1) * P, j * NFREE : (j + 1) * NFREE],
                )
                w_tiles[k][j] = wt

            def load_d0(eng, k, j):
                dt_t = dp.tile([P, NFREE], mybir.dt.bfloat16, tag=f"d0_{k}_{j}")
                eng.dma_start(
                    out=dt_t[:],
                    in_=dT[k * P : (k + 1) * P, j * NFREE : (j + 1) * NFREE],
                )
                d0[k][j] = dt_t

            # Primer triple for the ramp's k=0 round, on three parallel
            # queues so the first real matmuls start as early as possible.
            load_w(nc.sync, 0, 0)
            load_d0(nc.scalar, 0, 0)
            load_w(nc.gpsimd, 0, 1)

            # Remaining loads in the exact order the k-major ramp consumes
            # them, alternated across the two HWDGE queues.
            loads = []
            for k in range(1, KT):
                loads.append(("w", k, 0))
                loads.append(("w", k, 1))
                loads.append(("d0", k, 0))
            loads.append(("bias", 0, 0))
            for k in range(KT):
                loads.append(("d0", k, 1))
            for c in range(1, NCHUNKS):
                for k in range(KT):
                    loads.append(("d", k, c))

            bias_t = None
            for i, (kind, k, j) in enumerate(loads):
                eng = nc.scalar if i % 2 == 0 else nc.sync
                if kind == "w":
                    load_w(eng, k, j)
                elif kind == "bias":
                    bias_t = bp.tile([P, OUT_DIM], mybir.dt.float32)
                    eng.dma_start(out=bias_t[:], in_=biasb[:, :])
                elif kind == "d0":
                    load_d0(eng, k, j)
                else:
                    dt_t = dp.tile([P, CCHUNK], mybir.dt.bfloat16, tag=f"d{k}_{j}")
                    eng.dma_start(
                        out=dt_t[:],
                        in_=dT[k * P : (k + 1) * P, j * CCHUNK : (j + 1) * CCHUNK],
                    )
                    d_tiles[k][j] = dt_t

            def sub_lhsT(k, sub):
                if sub < 4:
                    return d0[k][0][:, sub * P : (sub + 1) * P]
                if sub < 8:
                    return d0[k][1][:, (sub - 4) * P : (sub - 3) * P]
                c = sub // SUBS_PER_CHUNK
                s = sub - c * SUBS_PER_CHUNK
                return d_tiles[k][c][:, s * P : (s + 1) * P]

            def evac_half(sub, h, ps, split=False):
                """Bias-add ps into a bf16 tile and store out[sub, half h]."""
                r0 = sub * P
                c0 = h * NFREE
                if sub < 8:
                    # Early stores on gpsimd (software DGE: slow, but their
                    # completion is latency-insensitive mid-kernel); HWDGE
                    # queues stay free for the primer/chunk loads.
                    ot = op.tile([P, NFREE], mybir.dt.bfloat16, tag="ot")
                    nc.vector.tensor_add(ot[:], ps[:], bias_t[:, c0 : c0 + NFREE])
                    nc.gpsimd.dma_start(
                        out=out[r0 : r0 + P, c0 : c0 + NFREE], in_=ot[:]
                    )
                elif split:
                    # Final store: two 256-col pieces on both HWDGE queues so
                    # the end-of-kernel drain is as short as possible.
                    H = NFREE // 2
                    for q, eng in enumerate((nc.sync, nc.scalar)):
                        ot = op.tile([P, H], mybir.dt.bfloat16, tag=f"otq{q}")
                        nc.vector.tensor_add(
                            ot[:],
                            ps[:, q * H : (q + 1) * H],
                            bias_t[:, c0 + q * H : c0 + (q + 1) * H],
                        )
                        eng.dma_start(
                            out=out[r0 : r0 + P, c0 + q * H : c0 + (q + 1) * H],
                            in_=ot[:],
                        )
                else:
                    ot = op.tile([P, NFREE], mybir.dt.bfloat16, tag="ot")
                    nc.vector.tensor_add(ot[:], ps[:], bias_t[:, c0 : c0 + NFREE])
                    eng = nc.sync if (2 * sub + h) % 2 == 0 else nc.scalar
                    eng.dma_start(out=out[r0 : r0 + P, c0 : c0 + NFREE], in_=ot[:])

            # Ramp psum banks: 4 subs x 2 halves = all 8 banks.
            ramp = [
                (pp.tile([P, NFREE], mybir.dt.float32, tag="ps0", name=f"rps0_{s}"),
                 pp.tile([P, NFREE], mybir.dt.float32, tag="ps1", name=f"rps1_{s}"))
                for s in range(4)
            ]

            # PE warm-up: the PE is DMA-idle until the primer loads land, so
            # its HAM clock gate would hold it at 1.2 GHz for the first
            # ~3.4 us of real work. Run small dummy matmuls on the zeroed
            # scratch tile so the clock is warming while loads stream in.
            # Target ramp[3][1]: the last bank the real ramp touches, so the
            # WAW dependency never stalls the first real matmuls.
            for wi in range(28):
                nc.tensor.matmul(
                    ramp[3][1][:, 0:P], scratch[:], scratch[:],
                    start=True, stop=True,
                )

            # Ramp: k-major over the first 4 subtiles (8 PSUM banks live) so
            # each arriving (w[k], d0a[k]) pair unlocks 8 matmuls.
            for k in range(KT):
                for s in range(4):
                    lhsT = sub_lhsT(k, s)
                    nc.tensor.matmul(
                        ramp[s][0][:], lhsT, w_tiles[k][0][:],
                        start=(k == 0), stop=(k == KT - 1),
                    )
                    nc.tensor.matmul(
                        ramp[s][1][:], lhsT, w_tiles[k][1][:],
                        start=(k == 0), stop=(k == KT - 1),
                    )
            for s in range(4):
                evac_half(s, 0, ramp[s][0])
                evac_half(s, 1, ramp[s][1])

            # Steady state: sub-major, each 512-wide output half as its own
            # contiguous k-run so its evac+store overlaps the next k-run.
            for sub in range(4, NSUBS):
                last = sub == NSUBS - 1
                for h in range(2):
                    ps = pp.tile([P, NFREE], mybir.dt.float32, tag=f"ps{h}")
                    for k in range(KT):
                        nc.tensor.matmul(
                            ps[:], sub_lhsT(k, sub), w_tiles[k][h][:],
                            start=(k == 0), stop=(k == KT - 1),
                        )
                    evac_half(sub, h, ps, split=(last and h == 1))

    nc.compile()
    return nc


def _get_nc():
    if "nc" not in _CACHE:
        _CACHE["nc"] = _build()
    return _CACHE["nc"]


def _prep_inputs(data, W, b):
    data = np.asarray(data, dtype=np.float32)
    W = np.asarray(W, dtype=np.float32)
    b = np.asarray(b, dtype=np.float32)
    wT = np.ascontiguousarray(W.astype(ml_dtypes.bfloat16).T)  # [in, out] bf16
    bias_bc = np.ascontiguousarray(
        np.broadcast_to(b[None, :], (P, OUT_DIM))
    )  # [128, 1024] f32
    in_maps = []
    for c in range(N_CORES):
        shard = data[c * SHARD : (c + 1) * SHARD]  # [4096, 1024] f32
        dT = np.ascontiguousarray(shard.astype(ml_dtypes.bfloat16).T)  # [in, batch]
        in_maps.append({"dT": dT, "wT": wT, "biasb": bias_bc})
    return in_maps


def _run(data, W, b, trace=False, **trace_kw):
    nc = _get_nc()
    in_maps = _prep_inputs(data, W, b)
    res = run_bass_kernel_spmd(nc, in_maps, list(range(N_CORES)), trace=trace, **trace_kw)
    out = np.concatenate(
        [
            np.asarray(res.results[c]["out"]).astype(np.float32)
            for c in range(N_CORES)
        ],
        axis=0,
    )
    return out, res


def kernel(**inputs) -> np.ndarray:
    out, _ = _run(inputs["data"], inputs["W"], inputs["b"])
    return out


# revision 3
# speedup vs baseline: 1.1756x; 1.1756x over previous
"""GroupFC kernel for Trainium2, data-parallel across 8 NeuronCores.

Problem: out = data @ W.T + b
  data: [32768, 1024] f32, W: [1024, 1024] f32 (block-diagonal-masked), b: [1024] f32

Strategy:
  - Shard batch dim across 8 cores (4096 rows each); replicate W, b.
  - Host-side: cast data shard + W to bf16, pre-transpose so the contraction
    dim (in_features) lands on SBUF partitions; broadcast b to [128, 1024].
  - On-chip per core: the whole 8 MiB bf16 data shard is SBUF-resident as
    64 independent chunk tiles (fine-grained deps let the PE start as soon
    as the first chunks land). out_tile[128b, 512o] is accumulated over 8
    K-tiles in PSUM (bf16 operands, fp32 accumulate). Each data k-block is
    the PE-stationary operand shared by the two 512-wide output halves
    (one LDWEIGHTS per two matmuls -- the weight load is fully hidden and
    the 512 matmuls run at the 216 ns/MM hardware floor). Bias is added
    during PSUM->SBUF evacuation on DVE with bf16 output (halves store
    traffic; host upcasts to f32); stores go out in natural [batch, out]
    layout. The last sub-tile's output is evacuated and stored as four
    256-col pieces across both HWDGE queues to minimize the end-of-kernel
    drain.
"""

import os
import sys
from contextlib import ExitStack

import numpy as np

try:
    import concourse.bass as bass  # noqa: F401
except ImportError:
    sys.path.insert(0, "/opt/trn_rl_repo")

import ml_dtypes

import concourse.tile as tile
from concourse import bacc, mybir
from concourse.bass_utils import run_bass_kernel_spmd

N_CORES = 8
BATCH = 32768
SHARD = BATCH // N_CORES  # 4096
IN_DIM = 1024
OUT_DIM = 1024
P = 128
KT = IN_DIM // P  # 8 contraction tiles
NFREE = 512  # psum bank free-dim (fp32)
CCHUNK = 1024  # batch columns per data chunk tile
NCHUNKS = SHARD // CCHUNK  # 4
SUBS_PER_CHUNK = CCHUNK // P  # 8
NSUBS = SHARD // P  # 32

_CACHE = {}


def _build():
    nc = bacc.Bacc("TRN2", target_bir_lowering=False, debug=False)
    dT = nc.dram_tensor(
        "dT", [IN_DIM, SHARD], mybir.dt.bfloat16, kind="ExternalInput"
    ).ap()
    wT = nc.dram_tensor(
        "wT", [IN_DIM, OUT_DIM], mybir.dt.bfloat16, kind="ExternalInput"
    ).ap()
    biasb = nc.dram_tensor(
        "biasb", [P, OUT_DIM], mybir.dt.float32, kind="ExternalInput"
    ).ap()
    out = nc.dram_tensor(
        "out", [SHARD, OUT_DIM], mybir.dt.bfloat16, kind="ExternalOutput"
    ).ap()

    with tile.TileContext(nc) as tc:
        with ExitStack() as ctx:
            wp = ctx.enter_context(tc.tile_pool(name="w", bufs=1))
            bp = ctx.enter_context(tc.tile_pool(name="bias", bufs=1))
            dp = ctx.enter_context(tc.tile_pool(name="d", bufs=1))
            pp = ctx.enter_context(tc.tile_pool(name="psum", bufs=4, space="PSUM"))
            op = ctx.enter_context(tc.tile_pool(name="o", bufs=8))

            # Scratch for PE warm-up, memset early so dummies start right
            # after the framework preamble.
            scratch = wp.tile([P, P], mybir.dt.bfloat16, tag="warm_scratch")
            nc.vector.memset(scratch[:], 0)

            # w_tiles[k][nh]: [128, 512] halves of wT k-tile.
            w_tiles = [[None] * 2 for _ in range(KT)]
            # d0a/d0b: first chunk split as two [128, 512] tiles (subs 0-3 /
            # 4-7); d_tiles[k][c] for c>=1: [128, 1024] chunks (8 subs each).
            d0 = [[None] * 2 for _ in range(KT)]
            d_tiles = [[None] * NCHUNKS for _ in range(KT)]

            # Load plan: small primer transfers first, in the exact order the
            # k-major ramp consumes them, alternated across two load queues.
            loads = [("w", 0, 0), ("d0", 0, 0), ("w", 0, 1)]
            for k in range(1, KT):
                loads.append(("w", k, 0))
                loads.append(("w", k, 1))
                loads.append(("d0", k, 0))
            loads.append(("bias", 0, 0))
            for k in range(KT):
                loads.append(("d0", k, 1))
            for c in range(1, NCHUNKS):
                for k in range(KT):
                    loads.append(("d", k, c))

            bias_t = None
            for i, (kind, k, j) in enumerate(loads):
                eng = nc.scalar if i % 2 == 0 else nc.sync
                if kind == "w":
                    wt = wp.tile([P, NFREE], mybir.dt.bfloat16, tag=f"w{k}_{j}")
                    eng.dma_start(
                        out=wt[:],
                        in_=wT[k * P : (k + 1) * P, j * NFREE : (j + 1) * NFREE],
                    )
                    w_tiles[k][j] = wt
                elif kind == "bias":
                    bias_t = bp.tile([P, OUT_DIM], mybir.dt.float32)
                    eng.dma_start(out=bias_t[:], in_=biasb[:, :])
                elif kind == "d0":
                    dt_t = dp.tile([P, NFREE], mybir.dt.bfloat16, tag=f"d0_{k}_{j}")
                    eng.dma_start(
                        out=dt_t[:],
                        in_=dT[k * P : (k + 1) * P, j * NFREE : (j + 1) * NFREE],
                    )
                    d0[k][j] = dt_t
                else:
                    dt_t = dp.tile([P, CCHUNK], mybir.dt.bfloat16, tag=f"d{k}_{j}")
                    eng.dma_start(
                        out=dt_t[:],
                        in_=dT[k * P : (k + 1) * P, j * CCHUNK : (j + 1) * CCHUNK],
                    )
                    d_tiles[k][j] = dt_t

            def sub_lhsT(k, sub):
                if sub < 4:
                    return d0[k][0][:, sub * P : (sub + 1) * P]
                if sub < 8:
                    return d0[k][1][:, (sub - 4) * P : (sub - 3) * P]
                c = sub // SUBS_PER_CHUNK
                s = sub - c * SUBS_PER_CHUNK
                return d_tiles[k][c][:, s * P : (s + 1) * P]

            def evacuate(sub, ps0, ps1):
                r0 = sub * P
                if sub < 8:
                    # Early stores on gpsimd (software DGE: slow, but their
                    # completion is latency-insensitive mid-kernel), keeping
                    # the HWDGE queues free for the primer/chunk loads.
                    ot = op.tile([P, OUT_DIM], mybir.dt.bfloat16, tag="ot")
                    nc.vector.tensor_add(ot[:, 0:NFREE], ps0[:], bias_t[:, 0:NFREE])
                    nc.vector.tensor_add(
                        ot[:, NFREE:OUT_DIM], ps1[:], bias_t[:, NFREE:OUT_DIM]
                    )
                    nc.gpsimd.dma_start(out=out[r0 : r0 + P, :], in_=ot[:])
                elif sub < NSUBS - 1:
                    # HWDGE queues are past the load backlog here; store the
                    # two halves on alternating queues.
                    ot = op.tile([P, OUT_DIM], mybir.dt.bfloat16, tag="ot")
                    nc.vector.tensor_add(ot[:, 0:NFREE], ps0[:], bias_t[:, 0:NFREE])
                    nc.vector.tensor_add(
                        ot[:, NFREE:OUT_DIM], ps1[:], bias_t[:, NFREE:OUT_DIM]
                    )
                    e0 = nc.scalar if sub % 2 == 0 else nc.sync
                    e1 = nc.sync if sub % 2 == 0 else nc.scalar
                    e0.dma_start(out=out[r0 : r0 + P, 0:NFREE], in_=ot[:, 0:NFREE])
                    e1.dma_start(
                        out=out[r0 : r0 + P, NFREE:OUT_DIM], in_=ot[:, NFREE:OUT_DIM]
                    )
                else:
                    # Final sub: four 256-col pieces so the first store issues
                    # ~one DVE-add after the last matmul and the end-of-kernel
                    # drain is one small transfer per queue.
                    H = NFREE // 2
                    for q in range(4):
                        ps = ps0 if q < 2 else ps1
                        c0 = q * H
                        ot = op.tile([P, H], mybir.dt.bfloat16, tag=f"otq{q % 2}")
                        nc.vector.tensor_add(
                            ot[:], ps[:, (q % 2) * H : (q % 2 + 1) * H],
                            bias_t[:, c0 : c0 + H],
                        )
                        eng = nc.sync if q % 2 == 0 else nc.scalar
                        eng.dma_start(out=out[r0 : r0 + P, c0 : c0 + H], in_=ot[:])

            # Ramp psum banks: 4 subs x 2 halves = all 8 banks.
            ramp = [
                (pp.tile([P, NFREE], mybir.dt.float32, tag="ps0", name=f"rps0_{s}"),
                 pp.tile([P, NFREE], mybir.dt.float32, tag="ps1", name=f"rps1_{s}"))
                for s in range(4)
            ]

            # PE warm-up: the PE is DMA-idle until the primer loads land, so
            # its HAM clock gate would hold it at 1.2 GHz for the first
            # ~3.4 us of real work. Run small dummy matmuls on the zeroed
            # scratch tile so the clock is warm when real work starts.
            # Target ramp[3][1]: the last bank the real ramp touches, so the
            # WAW dependency never stalls the first real matmuls.
            for wi in range(28):
                nc.tensor.matmul(
                    ramp[3][1][:, 0:P], scratch[:], scratch[:],
                    start=True, stop=True,
                )

            # Ramp: k-major over the first 4 subtiles (8 PSUM banks live) so
            # each arriving (w[k], d0a[k]) pair unlocks 8 matmuls.
            for k in range(KT):
                for s in range(4):
                    lhsT = sub_lhsT(k, s)
                    nc.tensor.matmul(
                        ramp[s][0][:], lhsT, w_tiles[k][0][:],
                        start=(k == 0), stop=(k == KT - 1),
                    )
                    nc.tensor.matmul(
                        ramp[s][1][:], lhsT, w_tiles[k][1][:],
                        start=(k == 0), stop=(k == KT - 1),
                    )
            for s in range(4):
                evacuate(s, ramp[s][0], ramp[s][1])

            # Steady state: sub-major; each k-block of data is the stationary
            # operand shared by both output halves.
            for sub in range(4, NSUBS):
                ps0 = pp.tile([P, NFREE], mybir.dt.float32, tag="ps0")
                ps1 = pp.tile([P, NFREE], mybir.dt.float32, tag="ps1")
                for k in range(KT):
                    lhsT = sub_lhsT(k, sub)
                    nc.tensor.matmul(
                        ps0[:], lhsT, w_tiles[k][0][:],
                        start=(k == 0), stop=(k == KT - 1),
                    )
                    nc.tensor.matmul(
                        ps1[:], lhsT, w_tiles[k][1][:],
                        start=(k == 0), stop=(k == KT - 1),
                    )
                evacuate(sub, ps0, ps1)

    nc.compile()
    return nc


def _get_nc():
    if "nc" not in _CACHE:
        _CACHE["nc"] = _build()
    return _CACHE["nc"]


def _prep_inputs(data, W, b):
    data = np.asarray(data, dtype=np.float32)
    W = np.asarray(W, dtype=np.float32)
    b = np.asarray(b, dtype=np.float32)
    wT = np.ascontiguousarray(W.astype(ml_dtypes.bfloat16).T)  # [in, out] bf16
    bias_bc = np.ascontiguousarray(
        np.broadcast_to(b[None, :], (P, OUT_DIM))
    )  # [128, 1024] f32
    in_maps = []
    for c in range(N_CORES):
        shard = data[c * SHARD : (c + 1) * SHARD]  # [4096, 1024] f32
        dT = np.ascontiguousarray(shard.astype(ml_dtypes.bfloat16).T)  # [in, batch]
        in_maps.append({"dT": dT, "wT": wT, "biasb": bias_bc})
    return in_maps


def _run(data, W, b, trace=False, **trace_kw):
    nc = _get_nc()
    in_maps = _prep_inputs(data, W, b)
    res = run_bass_kernel_spmd(nc, in_maps, list(range(N_CORES)), trace=trace, **trace_kw)
    out = np.concatenate(
        [
            np.asarray(res.results[c]["out"]).astype(np.float32)
            for c in range(N_CORES)
        ],
        axis=0,
    )
    return out, res


def kernel(**inputs) -> np.ndarray:
    out, _ = _run(inputs["data"], inputs["W"], inputs["b"])
    return out
